# revision 12
# baseline (speedup 1.0000x reference)
"""Trainium2 Bass kernel for nn_AttentionS2 (spherical self-attention).

Module: y = p_w @ softmax_k(q k^T / sqrt(hd) + log_quad_w[k]) v + p_b
with q/k/v = 1x1-conv projections of the same input (self-attention),
B=1, C=512, H=W=64 (4096 tokens), 8 heads, head_dim=64.

Sharding: one head per NeuronCore (8 cores).

Key structure (per core):
  * All matmul operands are bf16 (x and weights are converted host-side).
  * The additive log-quadrature bias is folded multiplicatively into v:
    exp(s*S + lqw_k) = qw_k * exp(s*S), with qw_k also replacing the ones
    column used for the softmax denominator.  The exp is therefore
    bias-free and an exp tile can be any (key-tile x query-span) block.
  * k and q projections share one combined stationary [wk | A*wq] so the
    projection runs at M=128 (full PE columns).  A = 128*log2(e)/8 is a
    Schraudolph pre-scale folded into wq: the S matmul then directly
    produces logits in bf16-pattern units.
  * S^T is computed in (key x query) orientation, K=64, 512-query steps:
    stationary k from qk_sb rows 0:64, moving q from a partition-shifted
    copy q2 (rows 0:64); 256 (qc, kt) steps of one S + one AV matmul.
  * exp alternates between two engines: ACT (table exp, out bf16) and a
    custom DVE op (corrected Schraudolph: int16 round-to-nearest of
    y - k*(128|a|-a^2)/128 where y = x + B yields the bf16 bit pattern
    of exp; max rel err ~0.7%).
  * AV accumulates [v'|qw]^T P in PSUM over 32 key tiles; two PSUM
    accumulators alternate across qc so the normalization chain of one
    chunk overlaps the next chunk's accumulation.  Normalization uses a
    fast approximate reciprocal + a K=1 ones-matmul broadcast.
  * All PSUM->SBUF drains (projection copies, accumulator copies, bias
    adds) run on the otherwise-idle Pool engine; DVE only does exp + the
    tiny reciprocal chain.
  * DMs are batched (one dma_start per logical tensor, strided/rearranged
    APs) and split across the SP (HWDGE) and Pool (SWDGE) issue paths so
    descriptor-generation overhead stays off the critical path.
  * The head->token reshard is 4 pipelined AllToAlls (one per pair of
    512-query chunks); each is followed by that token block's output
    projection, so all but the last exchange+projection overlap attention
    compute.  Core d ends up owning tokens {1024p + 512j + 64d + u}.
"""

import contextlib
import os
import sys
import types

import numpy as np
import ml_dtypes

import concourse.bass as bass
import concourse.bacc as bacc
import concourse.tile as tile
from concourse import mybir
from concourse import bass_utils

# This container has no axon NTFF profile hook; shim the module so
# run_bass_kernel_spmd(trace=True) degrades gracefully instead of raising.
try:  # pragma: no cover
    import antenv.axon_hooks  # noqa: F401
except Exception:  # ModuleNotFoundError, or antenv missing entirely
    try:
        import antenv  # noqa: F401
    except Exception:
        antenv_mod = types.ModuleType("antenv")
        sys.modules["antenv"] = antenv_mod
    shim = types.ModuleType("antenv.axon_hooks")
    shim.get_axon_ntff_profile_hook = lambda: None
    sys.modules["antenv.axon_hooks"] = shim

F32 = mybir.dt.float32
F32R = mybir.dt.float32r
BF16 = mybir.dt.bfloat16
I16 = mybir.dt.int16
AF = mybir.ActivationFunctionType

C = 512          # channels
T = 4096         # tokens (H*W)
HD = 64          # head dim
NCORES = 8
NKT = T // 128   # 32 key tiles of 128
QC = 512         # query chunk width for the attention inner loop
NQC = T // QC    # 8
NPAIR = NQC // 2  # 4 chunk pairs -> 4 AllToAlls
SCALE = 1.0 / float(np.sqrt(HD))
LOOKAHEAD = 3    # S matmuls emitted this many steps ahead of exp/AV

# Schraudolph constants: logits arrive pre-scaled by A (folded into wq),
# i.e. psum = A * (q.k) with A = 128*log2(e)*SCALE.  Then the bf16 bit
# pattern of exp(SCALE*q.k) is round(y + corr), y = psum + B.
A_PRE = float(128.0 * np.log2(np.e) * SCALE)
B_SCH = 16255.8
K_SCH = 0.335
C0_SCH = float(np.float32(B_SCH + 3.0 * 2.0**29))
ACT_SCALE = float(np.log(2.0) / 128.0)   # ACT exp: e^(ACT_SCALE * psum)

# packed f32 const columns: 0 bqk | 1 bv | 2:6 pb | 6:38 qwf
CPK_W = 6 + NKT

_CACHE = {}

# exp-engine split: step g -> DVE iff pattern[g % len] set.
import os as _os
_PATSEL = _os.environ.get("KERNEL_DVE_PAT", "std")
if _PATSEL == "none" or _os.environ.get("KERNEL_NO_DVE_EXP", "0") == "1":
    DVE_PAT = (0,)
elif _PATSEL == "all":
    DVE_PAT = (1,)
else:
    DVE_PAT = (0, 1)   # alternate ACT / DVE
_VARIANT = "notail" if _os.environ.get("KERNEL_NOTAIL", "0") == "1" else "full"


def _register_exp_op():
    """Register the corrected-Schraudolph exp custom DVE op (idempotent)."""
    from concourse import dve_ops as dvo
    from concourse.dve_spec import Spec, Src0, Src1, C0, C1, C2, lower, Bin, AluOp

    name = "SCHRAUDOLPH_EXP_BF16_ANT"
    for op in dvo.OPS:
        if op.name == name:
            return op
    # y = x + B; u = x + (B + 3*2^29) rounds to the 128 grid; v = u - 3*2^29
    # b = |y - v|; corr = b*(b-128)*(k/128);  out = y + corr  -> int16 RN
    y = Src0 + C1
    u = Src0 + C0
    v = u - (C0 - C1)
    b = Bin(AluOp.ABSOLUTE_DIFF, y, v)
    t = b * (b - C2)
    spec = Spec(body=y + t * Src1)
    row = dvo._CUSTOM_DVE_ROW_BASE + len(dvo.OPS)
    assert row < 0x20
    dvo._SUB_OPCODE_FOR_NAME[name] = row
    shas = {}
    for ver in ("v3", "v4"):
        compiled = bass_utils.DveOpSpec(
            name=name, opcode=row, uops=lower(spec, ver=ver), rd1_en=True)
        shas[ver] = compiled.sha(ver)
    op = dvo.DveOp(name, spec, subdim=False, uops_sha=shas)
    dvo.OPS.append(op)
    dvo.CUSTOM_DVE_SPECS[name] = spec
    return op


EXP_OP = _register_exp_op()


def _emit_body(nc, tc, io, rep):
    """Emit one full forward pass. `io` holds the DRAM tensor handles.

    Emission order software-pipelines the attention inner loop: the S
    matmuls run LOOKAHEAD iterations ahead of exp/AV.  Projections are
    interleaved into the qc==0 attention iterations so the first exp can
    start early while the rest of x is still loading.
    """
    x, wqk, wv, wp, cpk, qwb, onesr, y = io
    with contextlib.ExitStack() as ctx:
        big = ctx.enter_context(tc.tile_pool(name=f"big{rep}", bufs=1))
        wts = ctx.enter_context(tc.tile_pool(name=f"wts{rep}", bufs=1))
        vtp = ctx.enter_context(tc.tile_pool(name=f"vtp{rep}", bufs=1))
        ptlp = ctx.enter_context(tc.tile_pool(name=f"ptl{rep}", bufs=6))
        sml = ctx.enter_context(tc.tile_pool(name=f"sml{rep}", bufs=4))
        atp = ctx.enter_context(tc.tile_pool(name=f"atp{rep}", bufs=2))
        drp = ctx.enter_context(tc.tile_pool(name=f"drp{rep}", bufs=1, space="DRAM"))
        # PSUM: 4 (S staging) + 2 (av accumulators) + 1 (rb / projection
        # prefetch) + 1 (output projection) = 8 banks.
        pss = ctx.enter_context(tc.tile_pool(name=f"pss{rep}", bufs=4, space="PSUM"))
        psa = ctx.enter_context(tc.tile_pool(name=f"psa{rep}", bufs=2, space="PSUM"))
        psr = ctx.enter_context(tc.tile_pool(name=f"psr{rep}", bufs=1, space="PSUM"))
        psy = ctx.enter_context(tc.tile_pool(name=f"psy{rep}", bufs=1, space="PSUM"))

        # ---- weight/const loads (batched; split over SP + SWDGE paths) --
        wqk_sb = wts.tile([128, 4, 128], BF16, tag="wqk")
        wv_sb = wts.tile([128, 4, HD], BF16, tag="wv")
        wp_sb = wts.tile([128, 4, C], BF16, tag="wp")
        cpk_sb = wts.tile([128, CPK_W], F32, tag="cpk")
        qwb_sb = wts.tile([128, NKT], BF16, tag="qwb")
        onesr_sb = wts.tile([1, HD], F32R, tag="onesr")
        # full-size Src1 constant: [P,1]-broadcast Src1 crashes the DVE on
        # this silicon/runtime, so the k/128 constant is a full-width tile.
        ksch_sb = wts.tile([128, QC], F32, tag="ksch")

        x_sb = big.tile([128, 4, T], BF16, tag="x")

        def load_x_group(g, eng):
            sl = slice(512 * g, 512 * (g + 1))
            eng.dma_start(out=x_sb[:, :, sl],
                          in_=x[:, sl].rearrange("(c p) t -> p c t", c=4))

        load_x_group(0, nc.sync)
        nc.gpsimd.dma_start(out=wqk_sb,
                            in_=wqk.rearrange("(c p) m -> p c m", c=4))
        load_x_group(1, nc.sync)
        nc.gpsimd.dma_start(out=cpk_sb, in_=cpk[:, :])
        nc.gpsimd.dma_start(out=qwb_sb, in_=qwb[:, :])
        load_x_group(2, nc.sync)
        nc.sync.dma_start(out=wv_sb, in_=wv.rearrange("(c p) m -> p c m", c=4))
        nc.sync.dma_start(out=onesr_sb, in_=onesr[:, :])
        nc.gpsimd.memset(ksch_sb, K_SCH / 128.0)

        # qk_sb: k on rows 0:64, A*q on rows 64:128.  q2: partition-shifted
        # copy of the q half (rows 0:64) so S can run k-stationary/q-moving.
        qk_sb = big.tile([128, T], BF16, tag="qk")
        q2 = big.tile([HD, T], BF16, tag="q2")
        # token-major v' tiles: qw-scaled v plus the qw column (denominator)
        vt_all = vtp.tile([128, NKT, HD + 1], BF16, tag="vt")
        nc.gpsimd.tensor_copy(out=vt_all[:, :, HD], in_=qwb_sb)

        proj_par = [0]

        def proj_ps():
            # alternate projection-psum source between the two pools so
            # back-to-back projections don't serialize on one bank.
            proj_par[0] ^= 1
            pool = pss if proj_par[0] else psr
            return pool.tile([128, QC], F32, name="pps",
                             tag="ss" if pool is pss else "rb")

        def emit_qk_chunk(n):
            # matmul part; returns the PSUM drain as a closure so the caller
            # can emit it behind the step's exp (keeps ACT's queue head free).
            sl = slice(512 * n, 512 * (n + 1))
            ps = proj_ps()
            for ci in range(4):
                nc.tensor.matmul(ps, wqk_sb[:, ci, :], x_sb[:, ci, sl],
                                 start=(ci == 0), stop=(ci == 3))

            def drain():
                nc.scalar.activation(out=qk_sb[:, sl], in_=ps,
                                     func=AF.Identity, bias=cpk_sb[:, 0:1])
                nc.sync.dma_start(out=q2[:, sl], in_=qk_sb[HD:128, sl])
            return drain

        def emit_vt(t):
            ps = proj_ps()
            for ci in range(4):
                nc.tensor.matmul(ps[:, 0:HD],
                                 x_sb[:, ci, 128 * t:128 * (t + 1)],
                                 wv_sb[:, ci, :],
                                 start=(ci == 0), stop=(ci == 3))

            def drain():
                nc.scalar.activation(out=vt_all[:, t, 0:HD],
                                     in_=ps[:, 0:HD], func=AF.Identity,
                                     scale=cpk_sb[:, 6 + t:7 + t])
            return drain

        emit_qk_chunk(0)()
        emit_qk_chunk(1)()
        for t in range(8):
            emit_vt(t)()

        # ---- attention (flat software pipeline over (qc, kt)) ----------
        snd, rcv = [], []
        for p in range(NPAIR):
            snd.append(drp.tile([NCORES, HD, 128], BF16, tag=f"snd{p}",
                                name=f"snd{p}"))
            rcv.append(drp.tile([NCORES, HD, 128], BF16, tag=f"rcv{p}",
                                name=f"rcv{p}"))

        # interleaved projection/load work during qc==0, keyed by step kt.
        prefetch = {
            0: [("xg", 3, nc.gpsimd), ("qk", 2)],
            1: [("xg", 4, nc.sync), ("vt", 8), ("vt", 9)],
            2: [("vt", 10), ("vt", 11)],
            3: [("qk", 3)],
            4: [("xg", 5, nc.gpsimd), ("vt", 12), ("vt", 13)],
            5: [("vt", 14), ("vt", 15)],
            6: [("qk", 4)],
            7: [("xg", 6, nc.sync), ("vt", 16), ("vt", 17)],
            8: [("vt", 18), ("vt", 19)],
            9: [("qk", 5)],
            10: [("xg", 7, nc.gpsimd), ("vt", 20), ("vt", 21)],
            11: [("vt", 22), ("vt", 23)],
            12: [("qk", 6), ("wp",)],
            13: [("vt", 24), ("vt", 25)],
            14: [("vt", 26), ("vt", 27)],
            15: [("qk", 7)],
            16: [("vt", 28), ("vt", 29)],
            17: [("vt", 30), ("vt", 31)],
        }

        ss_tiles = {}

        def emit_s(qc, kt):
            ss = pss.tile([128, QC], F32, tag="ss")
            ss_tiles[(qc, kt)] = ss
            nc.tensor.matmul(ss, qk_sb[0:HD, 128 * kt:128 * (kt + 1)],
                             q2[:, QC * qc:QC * (qc + 1)],
                             start=True, stop=True)

        steps = [(qc, kt) for qc in range(NQC) for kt in range(NKT)]
        for i in range(LOOKAHEAD):
            emit_s(*steps[i])
        av = None
        for g, (qc, kt) in enumerate(steps):
            drains = []
            if qc == 0:
                for item in prefetch.get(kt, ()):
                    if item[0] == "xg":
                        load_x_group(item[1], item[2])
                    elif item[0] == "qk":
                        drains.append(emit_qk_chunk(item[1]))
                    elif item[0] == "vt":
                        drains.append(emit_vt(item[1]))
                    elif item[0] == "wp":
                        nc.sync.dma_start(
                            out=wp_sb,
                            in_=wp.rearrange("(c p) m -> p c m", c=4))
            if kt == 0:
                av = psa.tile([HD + 1, QC], F32, tag="av", name=f"av{qc}")
            ss = ss_tiles.pop((qc, kt))
            pt = ptlp.tile([128, QC], I16, tag="pt")
            if DVE_PAT[g % len(DVE_PAT)]:
                nc.vector._custom_dve(EXP_OP, out=pt, in0=ss,
                                      in1=ksch_sb, s0=C0_SCH, s1=B_SCH,
                                      imm2=128.0)
            else:
                nc.scalar.activation(out=pt.bitcast(BF16), in_=ss,
                                     func=AF.Exp, scale=ACT_SCALE)
            if g + LOOKAHEAD < len(steps):
                emit_s(*steps[g + LOOKAHEAD])
            for d in drains:
                d()
            nc.tensor.matmul(av, vt_all[:, kt, :], pt.bitcast(BF16),
                             start=(kt == 0), stop=(kt == NKT - 1),
                             skip_group_check=True)
            if kt == NKT - 1:
                # normalize: rows 0:64 numerator, row 64 denominator.  The
                # denominator is copied to a base-partition-0 tile: custom
                # DVE ops (reciprocal_approx_fast) misread partition-offset
                # inputs, and engine APs must start at a 32-aligned
                # partition.  The copy runs on ACT, the rest on DVE.
                av_sb = sml.tile([HD, QC], F32, tag="avs")
                nc.vector.tensor_copy(out=av_sb, in_=av[0:HD, :])
                den_t = sml.tile([1, QC], F32, tag="den")
                nc.scalar.copy(out=den_t, in_=av[HD:HD + 1, :])
                rcp = sml.tile([1, QC], F32, tag="rcp")
                with nc.allow_low_precision(reason="~18-bit 1/den for softmax"):
                    nc.vector.reciprocal_approx_fast(out=rcp, in_=den_t)
                rb = psr.tile([HD, QC], F32, tag="rb", name=f"rb{qc}")
                nc.tensor.matmul(rb, onesr_sb, rcp.bitcast(F32R),
                                 start=True, stop=True)
                ohp = sml.tile([HD, QC], BF16, tag="oh")
                nc.vector.tensor_mul(out=ohp, in0=av_sb, in1=rb)
                nc.gpsimd.tensor_scalar_add(out=ohp, in0=ohp,
                                            scalar1=cpk_sb[0:HD, 1:2])
                p, j = qc // 2, qc % 2
                nc.sync.dma_start(
                    out=snd[p][:, :, 64 * j:64 * (j + 1)].rearrange(
                        "d p t -> p d t"),
                    in_=ohp.rearrange("p (d t) -> p d t", d=NCORES))
                if j == 1:
                    _emit_pair_tail(nc, tc, p, snd, rcv, wp_sb, cpk_sb,
                                    atp, sml, psy, y)


def _emit_pair_tail(nc, tc, p, snd, rcv, wp_sb, cpk_sb, atp, sml, psy, y):
    """Exchange pair p's token block and run its output projection."""
    if _VARIANT == "notail":
        # sim-only: stand in for the AllToAll with a local DRAM copy so
        # TimelineSim (no collectives) can model the full program.
        nc.sync.dma_start(out=rcv[p][:, :, :], in_=snd[p][:, :, :])
    else:
        nc.gpsimd.collective_compute(
            "AllToAll", mybir.AluOpType.bypass,
            replica_groups=[list(range(NCORES))],
            ins=[snd[p][:, :, :]], outs=[rcv[p][:, :, :]])
    at_p = atp.tile([128, 4, 128], BF16, tag="at")
    nc.sync.dma_start(
        out=at_p, in_=rcv[p].rearrange("(c h) p t -> (h p) c t", c=4))
    yps = psy.tile([128, 4, 128], F32, tag="ypsum")
    for m in range(4):
        for ci in range(4):
            nc.tensor.matmul(yps[:, m, :],
                             wp_sb[:, ci, 128 * m:128 * (m + 1)],
                             at_p[:, ci, :],
                             start=(ci == 0), stop=(ci == 3))
    yo = sml.tile([128, 4, 128], F32, tag="yo")
    for m in range(4):
        nc.scalar.activation(out=yo[:, m, :], in_=yps[:, m, :],
                             func=AF.Identity, bias=cpk_sb[:, 2 + m:3 + m])
    nc.sync.dma_start(
        out=y[:, 128 * p:128 * (p + 1)].rearrange("(m q) t -> q m t", m=4),
        in_=yo)


def _build(repeat=1):
    nc = bacc.Bacc("TRN2", target_bir_lowering=False, debug=False,
                   num_devices=NCORES)
    x = nc.dram_tensor("x", [C, T], BF16, kind="ExternalInput")
    wqk = nc.dram_tensor("wqk", [C, 128], BF16, kind="ExternalInput")
    wv = nc.dram_tensor("wv", [C, HD], BF16, kind="ExternalInput")
    wp = nc.dram_tensor("wp", [C, C], BF16, kind="ExternalInput")
    cpk = nc.dram_tensor("cpk", [128, CPK_W], F32, kind="ExternalInput")
    qwb = nc.dram_tensor("qwb", [128, NKT], BF16, kind="ExternalInput")
    onesr = nc.dram_tensor("onesr", [1, HD], F32R, kind="ExternalInput")
    y = nc.dram_tensor("y", [C, T // NCORES], F32, kind="ExternalOutput")
    io = (x, wqk, wv, wp, cpk, qwb, onesr, y)

    with tile.TileContext(nc) as tc:
        for rep in range(repeat):
            _emit_body(nc, tc, io, rep)

    nc.finalize()
    return nc


def _get_nc(repeat=1):
    key = ("nc", repeat)
    if key not in _CACHE:
        _CACHE[key] = _build(repeat)
    return _CACHE[key]


def _quad_weights():
    # Clenshaw-Curtis quadrature weights on the 64-lat equiangular grid,
    # as torch-harmonics' 'equiangular' grid: flipped, * 2*pi/nlon.
    n = 64
    N = n - 1
    theta = np.pi * np.arange(n) / N
    m = N // 2
    j = np.arange(1, m + 1)
    b = np.where(2 * j == N, 1.0, 2.0)
    S = (b / (4.0 * j**2 - 1.0))[None, :] * np.cos(
        2.0 * j[None, :] * theta[:, None])
    w = 1.0 - S.sum(axis=1)
    c = np.full(n, 2.0)
    c[0] = 1.0
    c[-1] = 1.0
    w = (c * w / N)[::-1].copy()
    qw = 2.0 * np.pi * w / 64.0                       # (nlat,)
    return np.tile(qw[:, None], (1, 64)).reshape(-1)  # (T,)


def _in_maps(query, q_w, q_b, k_w, k_b, v_w, v_b, p_w, p_b, log_quad_weights):
    bf = ml_dtypes.bfloat16
    x = np.ascontiguousarray(
        np.asarray(query, np.float32).reshape(C, T)).astype(bf)
    wp = np.ascontiguousarray(np.asarray(p_w, np.float32).T).astype(bf)
    pbm = np.asarray(p_b, np.float32).reshape(4, 128).T
    qw = np.exp(np.asarray(log_quad_weights, np.float32)).astype(np.float64)
    qwf = qw.reshape(NKT, 128).T.astype(np.float32)
    qwb = np.ascontiguousarray(qwf.astype(bf))
    maps = []
    for h in range(NCORES):
        hs = slice(HD * h, HD * (h + 1))
        wk_h = np.asarray(k_w, np.float32)[hs, :].T          # [C, 64]
        wq_h = np.asarray(q_w, np.float32)[hs, :].T * A_PRE  # [C, 64]
        wqk = np.ascontiguousarray(
            np.concatenate([wk_h, wq_h], axis=1)).astype(bf)
        cpk = np.zeros((128, CPK_W), np.float32)
        cpk[0:HD, 0] = np.asarray(k_b, np.float32)[hs]
        cpk[HD:128, 0] = np.asarray(q_b, np.float32)[hs] * A_PRE
        cpk[0:HD, 1] = np.asarray(v_b, np.float32)[hs]
        cpk[:, 2:6] = pbm
        cpk[:, 6:6 + NKT] = qwf
        maps.append(dict(
            x=x,
            wqk=wqk,
            wv=np.ascontiguousarray(
                np.asarray(v_w, np.float32)[hs, :].T).astype(bf),
            wp=wp,
            cpk=np.ascontiguousarray(cpk),
            qwb=qwb,
            onesr=np.ones((1, HD), np.float32),
        ))
    return maps


def _run(in_maps, repeat=1, **kw):
    nc = _get_nc(repeat)
    return bass_utils.run_bass_kernel_spmd(nc, in_maps, list(range(NCORES)), **kw)


def _assemble(results):
    # core d owns tokens {1024p + 512j + 64d + u}; its y holds them at
    # column 128p + 64j + u.
    full = np.empty((C, T), np.float32)
    for d in range(NCORES):
        yd = results[d]["y"]
        for p in range(NPAIR):
            for j in range(2):
                full[:, 1024 * p + 512 * j + 64 * d:
                     1024 * p + 512 * j + 64 * (d + 1)] = \
                    yd[:, 128 * p + 64 * j:128 * p + 64 * (j + 1)]
    return np.ascontiguousarray(full.reshape(1, C, 64, 64))


def kernel(**inputs):
    res = _run(_in_maps(**inputs))
    return _assemble(res.results)


# revision 22
# speedup vs baseline: 1.2153x; 1.2153x over previous
"""Trainium2 Bass kernel for nn_AttentionS2 (spherical self-attention).

Module: y = p_w @ softmax_k(q k^T / sqrt(hd) + log_quad_w[k]) v + p_b
with q/k/v = 1x1-conv projections of the same input (self-attention),
B=1, C=512, H=W=64 (4096 tokens), 8 heads, head_dim=64.

Sharding: one head per NeuronCore (8 cores).

Key structure (per core):
  * All matmul operands are bf16 (x and weights are converted host-side).
  * The additive log-quadrature bias is folded multiplicatively into v:
    exp(s*S + lqw_k) = qw_k * exp(s*S), with qw_k also replacing the ones
    column used for the softmax denominator.  The exp is therefore
    bias-free and an exp tile can be any (key-tile x query-span) block.
  * k and q projections share one combined stationary [wk | A*wq] so the
    projection runs at M=128 (full PE columns).  A = 128*log2(e)/8 is a
    Schraudolph pre-scale folded into wq: the S matmul then directly
    produces logits in bf16-pattern units.
  * S^T is computed in (key x query) orientation over 128 supersteps
    (chunk-pair x key-tile).  Each superstep issues the pair's two
    512-query S matmuls into disjoint PE half-arrays (tile_position
    (0,0) / (64,0) via the qk_sb / kq2 swapped layouts) where they run
    concurrently on HW, then two serial AV matmuls.
  * exp is split deterministically: the even chunk's tile on ACT (table
    exp, out bf16), the odd chunk's on a custom DVE op (corrected
    Schraudolph: int16 round-to-nearest of y - k*(128|a|-a^2)/128,
    y = x + B, yields the bf16 bit pattern of exp; max rel err ~0.7%).
  * AV accumulates [v'|qw]^T P into two PSUM accumulators (one per live
    chunk) over 32 key tiles; normalization uses a fast approximate
    reciprocal + a K=1 ones-matmul broadcast.
  * PSUM->SBUF drains run on ACT (fused per-partition bias/scale via
    activation Identity); Pool does only SBUF->SBUF work and SWDGE DMA
    issue (GPSIMD cannot touch PSUM on TRN2).
  * DMAs are batched (one dma_start per logical tensor, strided APs)
    and split across the SP (HWDGE) and Pool (SWDGE) issue paths.
  * The head->token reshard is NPAIR/EXG pipelined AllToAlls; each is
    followed by its token block's output projection, so all but the
    last exchange+projection overlap attention compute.  Core d ends up
    owning tokens {1024p + 512j + 64d + u} (columns 128p + 64j + u of
    its y slice).
"""

import contextlib
import os
import sys
import types

import numpy as np
import ml_dtypes

import concourse.bass as bass
import concourse.bacc as bacc
import concourse.tile as tile
from concourse import mybir
from concourse import bass_utils

# This container has no axon NTFF profile hook; shim the module so
# run_bass_kernel_spmd(trace=True) degrades gracefully instead of raising.
try:  # pragma: no cover
    import antenv.axon_hooks  # noqa: F401
except Exception:  # ModuleNotFoundError, or antenv missing entirely
    try:
        import antenv  # noqa: F401
    except Exception:
        antenv_mod = types.ModuleType("antenv")
        sys.modules["antenv"] = antenv_mod
    shim = types.ModuleType("antenv.axon_hooks")
    shim.get_axon_ntff_profile_hook = lambda: None
    sys.modules["antenv.axon_hooks"] = shim

F32 = mybir.dt.float32
F32R = mybir.dt.float32r
BF16 = mybir.dt.bfloat16
I16 = mybir.dt.int16
AF = mybir.ActivationFunctionType

C = 512          # channels
T = 4096         # tokens (H*W)
HD = 64          # head dim
NCORES = 8
NKT = T // 128   # 32 key tiles of 128
QC = 512         # query chunk width for the attention inner loop
NQC = T // QC    # 8
NPAIR = NQC // 2  # 4 chunk pairs -> 4 AllToAlls
SCALE = 1.0 / float(np.sqrt(HD))
LOOKAHEAD = 3    # S matmuls emitted this many steps ahead of exp/AV

# Schraudolph constants: logits arrive pre-scaled by A (folded into wq),
# i.e. psum = A * (q.k) with A = 128*log2(e)*SCALE.  Then the bf16 bit
# pattern of exp(SCALE*q.k) is round(y + corr), y = psum + B.
A_PRE = float(128.0 * np.log2(np.e) * SCALE)
B_SCH = 16255.8
K_SCH = 0.335
C0_SCH = float(np.float32(B_SCH + 3.0 * 2.0**29))
ACT_SCALE = float(np.log(2.0) / 128.0)   # ACT exp: e^(ACT_SCALE * psum)

# packed f32 const columns: 0 bqk | 1 bv | 2:6 pb | 6:38 qwf
CPK_W = 6 + NKT

_CACHE = {}

# exp-engine split: step g -> DVE iff pattern[g % len] set.
import os as _os
_PATSEL = _os.environ.get("KERNEL_DVE_PAT", "std")
if _PATSEL == "none" or _os.environ.get("KERNEL_NO_DVE_EXP", "0") == "1":
    DVE_PAT = (0,)
elif _PATSEL == "all":
    DVE_PAT = (1,)
else:
    DVE_PAT = (0, 1)   # alternate ACT / DVE
_VARIANT = "notail" if _os.environ.get("KERNEL_NOTAIL", "0") == "1" else "full"
# query-chunk pairs per AllToAll exchange: 1 -> 4 exchanges, 4 -> single.
EXG = int(_os.environ.get("KERNEL_EXG", "1"))
assert NPAIR % EXG == 0


def _register_exp_op():
    """Register the corrected-Schraudolph exp custom DVE op (idempotent)."""
    from concourse import dve_ops as dvo
    from concourse.dve_spec import Spec, Src0, Src1, C0, C1, C2, lower, Bin, AluOp

    name = "SCHRAUDOLPH_EXP_BF16_ANT"
    for op in dvo.OPS:
        if op.name == name:
            return op
    # y = x + B; u = x + (B + 3*2^29) rounds to the 128 grid; v = u - 3*2^29
    # b = |y - v|; corr = b*(b-128)*(k/128);  out = y + corr  -> int16 RN
    y = Src0 + C1
    u = Src0 + C0
    v = u - (C0 - C1)
    b = Bin(AluOp.ABSOLUTE_DIFF, y, v)
    t = b * (b - C2)
    spec = Spec(body=y + t * Src1)
    row = dvo._CUSTOM_DVE_ROW_BASE + len(dvo.OPS)
    assert row < 0x20
    dvo._SUB_OPCODE_FOR_NAME[name] = row
    shas = {}
    for ver in ("v3", "v4"):
        compiled = bass_utils.DveOpSpec(
            name=name, opcode=row, uops=lower(spec, ver=ver), rd1_en=True)
        shas[ver] = compiled.sha(ver)
    op = dvo.DveOp(name, spec, subdim=False, uops_sha=shas)
    dvo.OPS.append(op)
    dvo.CUSTOM_DVE_SPECS[name] = spec
    return op


EXP_OP = _register_exp_op()


def _emit_body(nc, tc, io, rep):
    """Emit one full forward pass. `io` holds the DRAM tensor handles.

    Emission order software-pipelines the attention inner loop: the S
    matmuls run LOOKAHEAD iterations ahead of exp/AV.  Projections are
    interleaved into the qc==0 attention iterations so the first exp can
    start early while the rest of x is still loading.
    """
    x, wqk, wv, wp, cpk, qwb, onesr, y = io
    with contextlib.ExitStack() as ctx:
        big = ctx.enter_context(tc.tile_pool(name=f"big{rep}", bufs=1))
        wts = ctx.enter_context(tc.tile_pool(name=f"wts{rep}", bufs=1))
        vtp = ctx.enter_context(tc.tile_pool(name=f"vtp{rep}", bufs=1))
        ptlp = ctx.enter_context(tc.tile_pool(name=f"ptl{rep}", bufs=6))
        sml = ctx.enter_context(tc.tile_pool(name=f"sml{rep}", bufs=6))
        atp = ctx.enter_context(tc.tile_pool(name=f"atp{rep}", bufs=2))
        drp = ctx.enter_context(tc.tile_pool(name=f"drp{rep}", bufs=1, space="DRAM"))
        # PSUM: 5 (S staging) + 2 (av accumulators, one per live query
        # chunk) + 1 (rb / projection prefetch / output projection) = 8.
        pss = ctx.enter_context(tc.tile_pool(name=f"pss{rep}", bufs=5, space="PSUM"))
        psa = ctx.enter_context(tc.tile_pool(name=f"psa{rep}", bufs=2, space="PSUM"))
        psr = ctx.enter_context(tc.tile_pool(name=f"psr{rep}", bufs=1, space="PSUM"))

        # ---- weight/const loads (batched; split over SP + SWDGE paths) --
        wqk_sb = wts.tile([128, 4, 128], BF16, tag="wqk")
        wv_sb = wts.tile([128, 4, HD], BF16, tag="wv")
        wp_sb = wts.tile([128, 4, C], BF16, tag="wp")
        cpk_sb = wts.tile([128, CPK_W], F32, tag="cpk")
        qwb_sb = wts.tile([128, NKT], BF16, tag="qwb")
        onesr_sb = wts.tile([1, HD], F32R, tag="onesr")
        # full-size Src1 constant: [P,1]-broadcast Src1 crashes the DVE on
        # this silicon/runtime, so the k/128 constant is a full-width tile.
        ksch_sb = wts.tile([128, QC], F32, tag="ksch")

        x_sb = big.tile([128, 4, T], BF16, tag="x")

        def load_x_group(g, eng):
            sl = slice(512 * g, 512 * (g + 1))
            eng.dma_start(out=x_sb[:, :, sl],
                          in_=x[:, sl].rearrange("(c p) t -> p c t", c=4))

        load_x_group(0, nc.sync)
        nc.gpsimd.dma_start(out=wqk_sb,
                            in_=wqk.rearrange("(c p) m -> p c m", c=4))
        load_x_group(1, nc.sync)
        nc.gpsimd.dma_start(out=cpk_sb, in_=cpk[:, :])
        nc.gpsimd.dma_start(out=qwb_sb, in_=qwb[:, :])
        load_x_group(2, nc.sync)
        nc.sync.dma_start(out=wv_sb, in_=wv.rearrange("(c p) m -> p c m", c=4))
        nc.sync.dma_start(out=onesr_sb, in_=onesr[:, :])
        nc.gpsimd.memset(ksch_sb, K_SCH / 128.0)

        # qk_sb: k on rows 0:64, A*q on rows 64:128.  kq2: the swapped copy
        # (q on rows 0:64, k on rows 64:128) so the two S sub-matmuls of a
        # chunk pair land in disjoint PE half-arrays (tile_position (0,0) and
        # (64,0)) and run concurrently.
        qk_sb = big.tile([128, T], BF16, tag="qk")
        kq2 = big.tile([128, T], BF16, tag="kq2")
        # token-major v' tiles: qw-scaled v plus the qw column (denominator)
        vt_all = vtp.tile([128, NKT, HD + 1], BF16, tag="vt")
        nc.gpsimd.tensor_copy(out=vt_all[:, :, HD], in_=qwb_sb)

        proj_par = [0]

        def proj_ps():
            # alternate projection-psum source between the two pools so
            # back-to-back projections don't serialize on one bank.
            proj_par[0] ^= 1
            pool = pss if proj_par[0] else psr
            return pool.tile([128, QC], F32, name="pps",
                             tag="ss" if pool is pss else "rb")

        def emit_qk_chunk(n):
            # matmul part; returns the PSUM drain as a closure so the caller
            # can emit it behind the step's exp (keeps ACT's queue head free).
            sl = slice(512 * n, 512 * (n + 1))
            ps = proj_ps()
            for ci in range(4):
                nc.tensor.matmul(ps, wqk_sb[:, ci, :], x_sb[:, ci, sl],
                                 start=(ci == 0), stop=(ci == 3))

            def drain():
                nc.scalar.activation(out=qk_sb[:, sl], in_=ps,
                                     func=AF.Identity, bias=cpk_sb[:, 0:1])
                nc.sync.dma_start(out=kq2[0:HD, sl], in_=qk_sb[HD:128, sl])
                nc.sync.dma_start(out=kq2[HD:128, sl], in_=qk_sb[0:HD, sl])
            return drain

        def emit_vt(t):
            ps = proj_ps()
            for ci in range(4):
                nc.tensor.matmul(ps[:, 0:HD],
                                 x_sb[:, ci, 128 * t:128 * (t + 1)],
                                 wv_sb[:, ci, :],
                                 start=(ci == 0), stop=(ci == 3))

            def drain():
                nc.scalar.activation(out=vt_all[:, t, 0:HD],
                                     in_=ps[:, 0:HD], func=AF.Identity,
                                     scale=cpk_sb[:, 6 + t:7 + t])
            return drain

        emit_qk_chunk(0)()
        emit_qk_chunk(1)()
        for t in range(8):
            emit_vt(t)()

        # ---- attention (flat software pipeline over (qc, kt)) ----------
        # EXG pairs of query chunks per AllToAll exchange.
        nexch = NPAIR // EXG
        snd, rcv = [], []
        for e in range(nexch):
            snd.append(drp.tile([NCORES, HD, 128 * EXG], BF16, tag=f"snd{e}",
                                name=f"snd{e}"))
            rcv.append(drp.tile([NCORES, HD, 128 * EXG], BF16, tag=f"rcv{e}",
                                name=f"rcv{e}"))

        # interleaved projection/load work during qc==0, keyed by step kt.
        prefetch = {
            0: [("xg", 3, nc.gpsimd), ("qk", 2)],
            1: [("xg", 4, nc.sync), ("vt", 8), ("vt", 9)],
            2: [("vt", 10), ("vt", 11)],
            3: [("qk", 3)],
            4: [("xg", 5, nc.gpsimd), ("vt", 12), ("vt", 13)],
            5: [("vt", 14), ("vt", 15)],
            6: [("qk", 4)],
            7: [("xg", 6, nc.sync), ("vt", 16), ("vt", 17)],
            8: [("vt", 18), ("vt", 19)],
            9: [("qk", 5)],
            10: [("xg", 7, nc.gpsimd), ("vt", 20), ("vt", 21)],
            11: [("vt", 22), ("vt", 23)],
            12: [("qk", 6), ("wp",)],
            13: [("vt", 24), ("vt", 25)],
            14: [("vt", 26), ("vt", 27)],
            15: [("qk", 7)],
            16: [("vt", 28), ("vt", 29)],
            17: [("vt", 30), ("vt", 31)],
        }

        ss_tiles = {}

        def emit_spair(pi, kt):
            # S for the pair's two query chunks in disjoint PE half-arrays:
            # sub a (even chunk) k-stationary from qk_sb rows 0:64, q moving
            # from kq2 rows 0:64 -> tile (0,0); sub b (odd chunk) k-copy
            # stationary from kq2 rows 64:128, q moving from qk_sb rows
            # 64:128 -> tile (64,0).  The two matmuls run concurrently.
            ks = slice(128 * kt, 128 * (kt + 1))
            qe = slice(QC * 2 * pi, QC * (2 * pi + 1))
            qo = slice(QC * (2 * pi + 1), QC * (2 * pi + 2))
            ss_a = pss.tile([128, QC], F32, tag="ss", name="ssa")
            nc.tensor.matmul(ss_a, qk_sb[0:HD, ks], kq2[0:HD, qe],
                             start=True, stop=True)
            ss_b = pss.tile([128, QC], F32, tag="ss", name="ssb")
            nc.tensor.matmul(ss_b, kq2[HD:128, ks], qk_sb[HD:128, qo],
                             start=True, stop=True)
            ss_tiles[(pi, kt)] = (ss_a, ss_b)

        def emit_norm_tail(qc, av):
            # normalize: rows 0:64 numerator, row 64 denominator.  The
            # denominator is copied to a base-partition-0 tile: custom
            # DVE ops (reciprocal_approx_fast) misread partition-offset
            # inputs, and engine APs must start at a 32-aligned
            # partition.  The copy runs on ACT, the rest on DVE.
            av_sb = sml.tile([HD, QC], F32, tag="avs", name=f"avs{qc}")
            nc.vector.tensor_copy(out=av_sb, in_=av[0:HD, :])
            den_t = sml.tile([1, QC], F32, tag="den", name=f"den{qc}")
            nc.scalar.copy(out=den_t, in_=av[HD:HD + 1, :])
            rcp = sml.tile([1, QC], F32, tag="rcp", name=f"rcp{qc}")
            with nc.allow_low_precision(reason="~18-bit 1/den for softmax"):
                nc.vector.reciprocal_approx_fast(out=rcp, in_=den_t)
            rcr = sml.tile([1, QC], F32R, tag="rcr", name=f"rcr{qc}")
            nc.vector.tensor_copy(out=rcr, in_=rcp)
            rb = psr.tile([HD, QC], F32, tag="rb", name=f"rb{qc}")
            nc.tensor.matmul(rb, onesr_sb, rcr, start=True, stop=True)
            ohp = sml.tile([HD, QC], BF16, tag="oh", name=f"oh{qc}")
            nc.vector.tensor_mul(out=ohp, in0=av_sb, in1=rb)
            nc.gpsimd.tensor_scalar_add(out=ohp, in0=ohp,
                                        scalar1=cpk_sb[0:HD, 1:2])
            e, j = qc // (2 * EXG), qc % (2 * EXG)
            nc.sync.dma_start(
                out=snd[e][:, :, 64 * j:64 * (j + 1)].rearrange(
                    "d p t -> p d t"),
                in_=ohp.rearrange("p (d t) -> p d t", d=NCORES))
            if j == 2 * EXG - 1:
                _emit_exchange_tail(nc, tc, e, snd, rcv, wp_sb, cpk_sb,
                                    atp, sml, psr, y)

        steps = [(pi, kt) for pi in range(NQC // 2) for kt in range(NKT)]
        for i in range(LOOKAHEAD):
            emit_spair(*steps[i])
        av_e = av_o = None
        for g, (pi, kt) in enumerate(steps):
            drains = []
            if pi == 0:
                for item in prefetch.get(kt, ()):
                    if item[0] == "xg":
                        load_x_group(item[1], item[2])
                    elif item[0] == "qk":
                        drains.append(emit_qk_chunk(item[1]))
                    elif item[0] == "vt":
                        drains.append(emit_vt(item[1]))
                    elif item[0] == "wp":
                        nc.sync.dma_start(
                            out=wp_sb,
                            in_=wp.rearrange("(c p) m -> p c m", c=4))
            if kt == 0:
                av_e = psa.tile([HD + 1, QC], F32, tag="av",
                                name=f"av{2 * pi}")
                av_o = psa.tile([HD + 1, QC], F32, tag="av",
                                name=f"av{2 * pi + 1}")
            ss_a, ss_b = ss_tiles.pop((pi, kt))
            pt_a = ptlp.tile([128, QC], I16, tag="pt", name="pta")
            nc.scalar.activation(out=pt_a.bitcast(BF16), in_=ss_a,
                                 func=AF.Exp, scale=ACT_SCALE)
            pt_b = ptlp.tile([128, QC], I16, tag="pt", name="ptb")
            nc.vector._custom_dve(EXP_OP, out=pt_b, in0=ss_b,
                                  in1=ksch_sb, s0=C0_SCH, s1=B_SCH,
                                  imm2=128.0)
            if g + LOOKAHEAD < len(steps):
                emit_spair(*steps[g + LOOKAHEAD])
            for d in drains:
                d()
            nc.tensor.matmul(av_e, vt_all[:, kt, :], pt_a.bitcast(BF16),
                             start=(kt == 0), stop=(kt == NKT - 1),
                             skip_group_check=True)
            nc.tensor.matmul(av_o, vt_all[:, kt, :], pt_b.bitcast(BF16),
                             start=(kt == 0), stop=(kt == NKT - 1),
                             skip_group_check=True)
            if kt == NKT - 1:
                emit_norm_tail(2 * pi, av_e)
                emit_norm_tail(2 * pi + 1, av_o)


def _emit_exchange_tail(nc, tc, e, snd, rcv, wp_sb, cpk_sb, atp, sml, psr, y):
    """Exchange e's token block (EXG pairs) and run its output projection."""
    if _VARIANT == "notail":
        # sim-only: stand in for the AllToAll with a local DRAM copy so
        # TimelineSim (no collectives) can model the full program.
        nc.sync.dma_start(out=rcv[e][:, :, :], in_=snd[e][:, :, :])
    else:
        nc.gpsimd.collective_compute(
            "AllToAll", mybir.AluOpType.bypass,
            replica_groups=[list(range(NCORES))],
            ins=[snd[e][:, :, :]], outs=[rcv[e][:, :, :]])
    for p in range(EXG * e, EXG * (e + 1)):
        pl = p - EXG * e
        at_p = atp.tile([128, 4, 128], BF16, tag="at", name=f"at{p}")
        nc.sync.dma_start(
            out=at_p,
            in_=rcv[e][:, :, 128 * pl:128 * (pl + 1)].rearrange(
                "(c h) p t -> (h p) c t", c=4))
        yps = psr.tile([128, 4, 128], F32, tag="rb", name=f"yps{p}")
        for m in range(4):
            for ci in range(4):
                nc.tensor.matmul(yps[:, m, :],
                                 wp_sb[:, ci, 128 * m:128 * (m + 1)],
                                 at_p[:, ci, :],
                                 start=(ci == 0), stop=(ci == 3))
        yo = sml.tile([128, 4, 128], F32, tag="yo", name=f"yo{p}")
        for m in range(4):
            nc.scalar.activation(out=yo[:, m, :], in_=yps[:, m, :],
                                 func=AF.Identity,
                                 bias=cpk_sb[:, 2 + m:3 + m])
        nc.sync.dma_start(
            out=y[:, 128 * p:128 * (p + 1)].rearrange("(m q) t -> q m t",
                                                      m=4),
            in_=yo)


def _build(repeat=1):
    nc = bacc.Bacc("TRN2", target_bir_lowering=False, debug=False,
                   num_devices=NCORES)
    x = nc.dram_tensor("x", [C, T], BF16, kind="ExternalInput")
    wqk = nc.dram_tensor("wqk", [C, 128], BF16, kind="ExternalInput")
    wv = nc.dram_tensor("wv", [C, HD], BF16, kind="ExternalInput")
    wp = nc.dram_tensor("wp", [C, C], BF16, kind="ExternalInput")
    cpk = nc.dram_tensor("cpk", [128, CPK_W], F32, kind="ExternalInput")
    qwb = nc.dram_tensor("qwb", [128, NKT], BF16, kind="ExternalInput")
    onesr = nc.dram_tensor("onesr", [1, HD], F32R, kind="ExternalInput")
    y = nc.dram_tensor("y", [C, T // NCORES], F32, kind="ExternalOutput")
    io = (x, wqk, wv, wp, cpk, qwb, onesr, y)

    with tile.TileContext(nc) as tc:
        for rep in range(repeat):
            _emit_body(nc, tc, io, rep)

    nc.finalize()
    return nc


def _get_nc(repeat=1):
    key = ("nc", repeat)
    if key not in _CACHE:
        _CACHE[key] = _build(repeat)
    return _CACHE[key]


def _quad_weights():
    # Clenshaw-Curtis quadrature weights on the 64-lat equiangular grid,
    # as torch-harmonics' 'equiangular' grid: flipped, * 2*pi/nlon.
    n = 64
    N = n - 1
    theta = np.pi * np.arange(n) / N
    m = N // 2
    j = np.arange(1, m + 1)
    b = np.where(2 * j == N, 1.0, 2.0)
    S = (b / (4.0 * j**2 - 1.0))[None, :] * np.cos(
        2.0 * j[None, :] * theta[:, None])
    w = 1.0 - S.sum(axis=1)
    c = np.full(n, 2.0)
    c[0] = 1.0
    c[-1] = 1.0
    w = (c * w / N)[::-1].copy()
    qw = 2.0 * np.pi * w / 64.0                       # (nlat,)
    return np.tile(qw[:, None], (1, 64)).reshape(-1)  # (T,)


def _in_maps(query, q_w, q_b, k_w, k_b, v_w, v_b, p_w, p_b, log_quad_weights):
    bf = ml_dtypes.bfloat16
    x = np.ascontiguousarray(
        np.asarray(query, np.float32).reshape(C, T)).astype(bf)
    wp = np.ascontiguousarray(np.asarray(p_w, np.float32).T).astype(bf)
    pbm = np.asarray(p_b, np.float32).reshape(4, 128).T
    qw = np.exp(np.asarray(log_quad_weights, np.float32)).astype(np.float64)
    qwf = qw.reshape(NKT, 128).T.astype(np.float32)
    qwb = np.ascontiguousarray(qwf.astype(bf))
    maps = []
    for h in range(NCORES):
        hs = slice(HD * h, HD * (h + 1))
        wk_h = np.asarray(k_w, np.float32)[hs, :].T          # [C, 64]
        wq_h = np.asarray(q_w, np.float32)[hs, :].T * A_PRE  # [C, 64]
        wqk = np.ascontiguousarray(
            np.concatenate([wk_h, wq_h], axis=1)).astype(bf)
        cpk = np.zeros((128, CPK_W), np.float32)
        cpk[0:HD, 0] = np.asarray(k_b, np.float32)[hs]
        cpk[HD:128, 0] = np.asarray(q_b, np.float32)[hs] * A_PRE
        cpk[0:HD, 1] = np.asarray(v_b, np.float32)[hs]
        cpk[:, 2:6] = pbm
        cpk[:, 6:6 + NKT] = qwf
        maps.append(dict(
            x=x,
            wqk=wqk,
            wv=np.ascontiguousarray(
                np.asarray(v_w, np.float32)[hs, :].T).astype(bf),
            wp=wp,
            cpk=np.ascontiguousarray(cpk),
            qwb=qwb,
            onesr=np.ones((1, HD), np.float32),
        ))
    return maps


def _run(in_maps, repeat=1, **kw):
    nc = _get_nc(repeat)
    return bass_utils.run_bass_kernel_spmd(nc, in_maps, list(range(NCORES)), **kw)


def _assemble(results):
    # core d owns tokens {1024p + 512j + 64d + u}; its y holds them at
    # column 128p + 64j + u.
    full = np.empty((C, T), np.float32)
    for d in range(NCORES):
        yd = results[d]["y"]
        for p in range(NPAIR):
            for j in range(2):
                full[:, 1024 * p + 512 * j + 64 * d:
                     1024 * p + 512 * j + 64 * (d + 1)] = \
                    yd[:, 128 * p + 64 * j:128 * p + 64 * (j + 1)]
    return np.ascontiguousarray(full.reshape(1, C, 64, 64))


def kernel(**inputs):
    res = _run(_in_maps(**inputs))
    return _assemble(res.results)
